# revision 19
# baseline (speedup 1.0000x reference)
"""Trainium2 Bass kernel for a single-layer attention module (RMSNorm + QKV +
RoPE + causal attention over a KV cache + output projection), tensor-parallel
over 8 NeuronCores (4 heads each), per-head AllGather of attention outputs,
and per-core output-column blocks of the final projection.

Head-interleaved schedule: the QKV projection matmuls of head h+1 are emitted
interleaved into the attention groups of head h so the PE never waits on the
scalar engine's exp. The softmax denominator is accumulated on the vector
engine (4 bf16 sub-accumulators per (head, sc)) and collapsed with a single
ones-matmul per (head, sc) instead of one denominator matmul per tile.
Causal work on the new-key tiles is restricted to the unmasked column range,
with a single constant 128x128 triangle mask for the diagonal blocks.

Self-contained: takes FULL inputs, returns the FULL [1024, 4096] f32 output.
"""

import sys
from collections import deque
from contextlib import ExitStack

sys.path.insert(0, "/opt/trn_rl_repo")

import numpy as np
import ml_dtypes

import concourse.bass as bass  # noqa: F401
import concourse.bacc as bacc
import concourse.tile as tile
from concourse.tile_rust import add_dep_helper
from concourse import mybir
from concourse import bass_utils

BF16 = ml_dtypes.bfloat16
F32 = np.float32

N_CORES = 8
D, H, HD, S, C = 4096, 32, 128, 1024, 2048
T = C + S          # 3072 total keys
HL = H // N_CORES  # 4 heads per core
OC = HL * HD       # 512 local attention features per core
NDK = D // 128     # 32 contraction tiles over D
NTC = C // 128     # 16 cache t-tiles
NTN = S // 128     # 8 new-key t-tiles
NT = NTC + NTN     # 24 t-tiles
EPS = 1e-6
THETA = 10000.0

bf = mybir.dt.bfloat16
f32 = mybir.dt.float32


def _attn_groups():
    """Group list per head: (ti, sc, lo, diag).

    lo = first valid column within the sc-half (columns < lo fully masked).
    diag = column (== lo) where a 128-wide triangle mask applies, or None.
    """
    out = []
    for ti in range(NT):
        for sc in range(2):
            if ti < NTC:
                out.append((ti, sc, 0, None))
                continue
            tn = ti - NTC
            dsc = tn // 4                      # half containing the diagonal
            if sc < dsc:
                continue                       # fully masked
            if sc == dsc:
                lo = 128 * (tn % 4)
                out.append((ti, sc, lo, lo))
            else:
                out.append((ti, sc, 0, None))
    return out


def _build_nc():
    nc = bacc.Bacc("TRN2", target_bir_lowering=False, debug=False,
                   num_devices=N_CORES)

    # ---- DRAM I/O ----
    xs_t = nc.dram_tensor("xs_t", [128, NDK * S], bf, kind="ExternalInput")
    wq_col = nc.dram_tensor("wq_col", [HL, 128, NDK * 128], bf, kind="ExternalInput")
    wk_col = nc.dram_tensor("wk_col", [HL, 128, NDK * 128], bf, kind="ExternalInput")
    wv_col = nc.dram_tensor("wv_col", [HL, 128, NDK * 128], bf, kind="ExternalInput")
    wo_blk = nc.dram_tensor("wo_blk", [HL, 128, 8 * OC], bf, kind="ExternalInput")
    ckt = nc.dram_tensor("ckt", [HL, 128, C], bf, kind="ExternalInput")
    cvr = nc.dram_tensor("cvr", [HL, 128, C], bf, kind="ExternalInput")
    cosT = nc.dram_tensor("cosT", [128, S], bf, kind="ExternalInput")
    sinT = nc.dram_tensor("sinT", [128, S], bf, kind="ExternalInput")
    triD = nc.dram_tensor("triD", [128, 128], bf, kind="ExternalInput")
    ones_d = nc.dram_tensor("ones_d", [128, 128], bf, kind="ExternalInput")
    id_d = nc.dram_tensor("id_d", [128, 128], bf, kind="ExternalInput")
    # y stored transposed ([outcol, s]); host transposes back
    y = nc.dram_tensor("y", [OC, S], f32, kind="ExternalOutput")
    DEBUG = False
    if DEBUG:
        dbg_qr = nc.dram_tensor("dbg_qr", [128, HL * S], bf, kind="ExternalOutput")
        dbg_kr = nc.dram_tensor("dbg_kr", [128, HL * S], bf, kind="ExternalOutput")
        dbg_v = nc.dram_tensor("dbg_v", [128, HL * S], bf, kind="ExternalOutput")
        dbg_at = nc.dram_tensor("dbg_at", [128, HL * S], bf, kind="ExternalOutput")
        dbg_rsq = nc.dram_tensor("dbg_rsq", [128, S], bf, kind="ExternalOutput")
        dbg_ag = [nc.dram_tensor(f"dbg_ag{h}", [128, 8 * S], bf,
                  kind="ExternalOutput") for h in range(HL)]

    groups = _attn_groups()
    # first/last participating group per sc (for PV start/stop flags)
    part = {sc: [g for g in groups if g[1] == sc] for sc in range(2)}
    pv_first = {sc: part[sc][0][0] for sc in range(2)}
    pv_last = {sc: part[sc][-1][0] for sc in range(2)}

    with tile.TileContext(nc) as tc:
        with (
            tc.tile_pool(name="const", bufs=1) as cpool,
            tc.tile_pool(name="qk", bufs=1) as qkpool,
            tc.tile_pool(name="kv", bufs=4) as kvpool,
            tc.tile_pool(name="exp", bufs=6) as epool,
            tc.tile_pool(name="acc", bufs=1) as accpool,
            tc.tile_pool(name="rec", bufs=2) as recpool,
            tc.tile_pool(name="dram", bufs=1, space="DRAM") as dpool,
        ):
            # persistent per-head results
            qr = qkpool.tile([128, HL * S], bf, name="qr")
            kr = qkpool.tile([128, HL * S], bf, name="kr")
            v_sb = qkpool.tile([128, HL * S], bf, name="v_sb")
            attnT = qkpool.tile([128, HL * S], bf, name="attnT")
            rsqT = qkpool.tile([128, S], bf, name="rsqT")
            cosp = qkpool.tile([128, S], bf, name="cosp")
            sinp = qkpool.tile([128, S], bf, name="sinp")
            ag_in = [dpool.tile([128, S], bf, name=f"ag_in{h}") for h in range(HL)]
            ag_out = [dpool.tile([N_CORES * 128, S], bf, name=f"ag_out{h}",
                                 addr_space="Shared") for h in range(HL)]

            psum_es = ExitStack()
            psA = psum_es.enter_context(
                tc.tile_pool(name="psA", bufs=2, space="PSUM"))
            psS = psum_es.enter_context(
                tc.tile_pool(name="psS", bufs=2, space="PSUM"))
            psO = psum_es.enter_context(
                tc.tile_pool(name="psO", bufs=2, space="PSUM"))
            psT = psum_es.enter_context(
                tc.tile_pool(name="psT", bufs=2, space="PSUM"))

            head_kv = {}   # h -> (ck_sb, cv_sb)
            ag_insts = {}  # h -> collective BassInstruction

            def lts_ltv(h, ti):
                ck_sb, cv_sb = head_kv[h]
                if ti < NTC:
                    return (ck_sb[:, ti * 128:(ti + 1) * 128],
                            cv_sb[:, ti * 128:(ti + 1) * 128])
                tn = ti - NTC
                return (kr[:, h * S + tn * 128: h * S + (tn + 1) * 128],
                        v_sb[:, h * S + tn * 128: h * S + (tn + 1) * 128])

            INTERLEAVE = True

            def pull(gen, n):
                if not INTERLEAVE:
                    return False
                for _ in range(n):
                    if next(gen, None) is None:
                        return False
                return True

            def drain(gen):
                for _ in gen:
                    pass

            with (
                tc.tile_pool(name="xs", bufs=32) as xpool,
                tc.tile_pool(name="sq", bufs=2) as sqpool,
                tc.tile_pool(name="wcol", bufs=2) as wpool,
                tc.tile_pool(name="hh", bufs=2) as hpool,
                tc.tile_pool(name="rope", bufs=2) as rpool,
                tc.tile_pool(name="vv", bufs=2) as vpool,
            ):
                # ---- tiny constants + xs, ordered so PE starts ~2us in ----
                ones_t = cpool.tile([128, 128], bf, name="ones_t")
                nc.sync.dma_start(ones_t[:], ones_d[:])
                xs_ch = []
                xc = xpool.tile([128, S], bf, name="xs_ch")
                nc.sync.dma_start(xc[:], xs_t[:, 0:S])
                xs_ch.append(xc)

                wcols = {}

                def emit_wcol(which, wsrc, h, half):
                    key = (which, h)
                    if half == 0:
                        wcols[key] = wpool.tile([128, NDK * 128], bf, name="wcol")
                    halfsz = NDK * 64
                    nc.sync.dma_start(
                        wcols[key][:, half * halfsz:(half + 1) * halfsz],
                        wsrc[h][:, half * halfsz:(half + 1) * halfsz])

                emit_wcol("q", wq_col, 0, 0)
                xc = xpool.tile([128, S], bf, name="xs_ch")
                nc.sync.dma_start(xc[:], xs_t[:, S:2 * S])
                xs_ch.append(xc)
                emit_wcol("q", wq_col, 0, 1)
                for dk in range(2, 6):
                    xc = xpool.tile([128, S], bf, name="xs_ch")
                    nc.sync.dma_start(xc[:], xs_t[:, dk * S:(dk + 1) * S])
                    xs_ch.append(xc)
                cos_t = cpool.tile([128, S], bf, name="cos_t")
                nc.sync.dma_start(cos_t[:], cosT[:])
                sin_t = cpool.tile([128, S], bf, name="sin_t")
                nc.sync.dma_start(sin_t[:], sinT[:])
                id_t = cpool.tile([128, 128], bf, name="id_t")
                nc.sync.dma_start(id_t[:], id_d[:])
                tri_t = cpool.tile([128, 128], bf, name="tri_t")
                nc.sync.dma_start(tri_t[:], triD[:])
                for dk in range(6, NDK):
                    xc = xpool.tile([128, S], bf, name="xs_ch")
                    nc.sync.dma_start(xc[:], xs_t[:, dk * S:(dk + 1) * S])
                    xs_ch.append(xc)

                def xs_half(dk, sc):
                    return xs_ch[dk][:, sc * 512:(sc + 1) * 512]

                def emit_ckcv(h):
                    ck_sb = kvpool.tile([128, C], bf, name="ck_sb")
                    nc.sync.dma_start(ck_sb[:], ckt[h])
                    cv_sb = kvpool.tile([128, C], bf, name="cv_sb")
                    nc.sync.dma_start(cv_sb[:], cvr[h])
                    head_kv[h] = (ck_sb, cv_sb)

                def emit_rope(h, which, hh):
                    # dst = hh*cos' + rot(hh)*sin' where cos'/sin' fold rsq
                    rot = rpool.tile([128, S], bf, name="rot")
                    nc.sync.dma_start(rot[0:64, :], hh[64:128, :])
                    nc.sync.dma_start(rot[64:128, :], hh[0:64, :])
                    ta = rpool.tile([128, S], bf, name="ta", bufs=1)
                    nc.vector.tensor_mul(ta[:], hh[:], cosp[:])
                    tb = rpool.tile([128, S], bf, name="tb", bufs=1)
                    nc.vector.tensor_mul(tb[:], rot[:], sinp[:])
                    dst = qr if which == "q" else kr
                    nc.vector.tensor_add(dst[:, h * S:(h + 1) * S], ta[:], tb[:])

                # ================= head 0 QKV (inline, with RMSNorm) ========
                hh_q = hpool.tile([128, S], bf, name="hh")
                stats = {sc: psS.tile([128, 512], f32, name="ps")
                         for sc in range(2)}
                qps = {sc: psA.tile([128, 512], f32, name="psp")
                       for sc in range(2)}
                wc = wcols[("q", 0)]
                nacc = None
                for dk in range(NDK):
                    # squares accumulated 4 dk-tiles at a time on vector,
                    # then one ones-matmul per (group-of-4, sc) on the PE
                    if dk % 4 == 0:
                        nacc = sqpool.tile([128, S], bf, name="nacc", bufs=2)
                        nc.vector.tensor_mul(nacc[:], xs_ch[dk][:],
                                             xs_ch[dk][:])
                    else:
                        sqt = sqpool.tile([128, S], bf, name="sqt", bufs=2)
                        nc.vector.tensor_mul(sqt[:], xs_ch[dk][:],
                                             xs_ch[dk][:])
                        nc.vector.tensor_add(nacc[:], nacc[:], sqt[:])
                    for sc in range(2):
                        if dk % 4 == 3:
                            nc.tensor.matmul(stats[sc][:], ones_t[:],
                                             nacc[:, sc * 512:(sc + 1) * 512],
                                             start=(dk == 3),
                                             stop=(dk == NDK - 1))
                        nc.tensor.matmul(qps[sc][:],
                                         wc[:, dk * 128:(dk + 1) * 128],
                                         xs_half(dk, sc),
                                         start=(dk == 0), stop=(dk == NDK - 1))
                for sc in range(2):
                    # rsq half + folded rope tables
                    ssum = recpool.tile([128, 512], f32, name="ssum", bufs=1)
                    nc.scalar.activation(ssum[:], stats[sc][:],
                                         mybir.ActivationFunctionType.Copy,
                                         bias=EPS, scale=1.0 / D)
                    rcp = recpool.tile([128, 512], f32, name="rcp", bufs=1)
                    nc.vector.reciprocal_approx_fast(rcp[:], ssum[:])
                    nc.scalar.sqrt(rsqT[:, sc * 512:(sc + 1) * 512], rcp[:])
                    nc.vector.tensor_mul(cosp[:, sc * 512:(sc + 1) * 512],
                                         cos_t[:, sc * 512:(sc + 1) * 512],
                                         rsqT[:, sc * 512:(sc + 1) * 512])
                    nc.vector.tensor_mul(sinp[:, sc * 512:(sc + 1) * 512],
                                         sin_t[:, sc * 512:(sc + 1) * 512],
                                         rsqT[:, sc * 512:(sc + 1) * 512])
                    nc.vector.tensor_copy(hh_q[:, sc * 512:(sc + 1) * 512],
                                          qps[sc][:])

                emit_ckcv(0)
                emit_wcol("k", wk_col, 0, 0)
                emit_wcol("k", wk_col, 0, 1)
                hh_k = hpool.tile([128, S], bf, name="hh")
                for sc in range(2):
                    kp = psA.tile([128, 512], f32, name="psp")
                    wc = wcols[("k", 0)]
                    for dk in range(NDK):
                        nc.tensor.matmul(kp[:], wc[:, dk * 128:(dk + 1) * 128],
                                         xs_half(dk, sc),
                                         start=(dk == 0), stop=(dk == NDK - 1))
                    nc.vector.tensor_copy(hh_k[:, sc * 512:(sc + 1) * 512], kp[:])
                emit_rope(0, "q", hh_q)
                emit_rope(0, "k", hh_k)

                emit_wcol("v", wv_col, 0, 0)
                emit_wcol("v", wv_col, 0, 1)
                vvt = vpool.tile([128, S], bf, name="vv")
                for sc in range(2):
                    vp = psA.tile([128, 512], f32, name="psp")
                    wc = wcols[("v", 0)]
                    for dk in range(NDK):
                        nc.tensor.matmul(vp[:], wc[:, dk * 128:(dk + 1) * 128],
                                         xs_half(dk, sc),
                                         start=(dk == 0), stop=(dk == NDK - 1))
                    nc.vector.tensor_mul(vvt[:, sc * 512:(sc + 1) * 512], vp[:],
                                         rsqT[:, sc * 512:(sc + 1) * 512])
                for tj in range(NTN):
                    ptr = psT.tile([128, 128], bf, name="ptr")
                    nc.tensor.transpose(ptr[:], vvt[:, tj * 128:(tj + 1) * 128],
                                        id_t[:])
                    nc.scalar.copy(v_sb[:, tj * 128:(tj + 1) * 128], ptr[:])

                # ============ QKV emission units for heads 1..3 =============
                def qkv_units(h):
                    emit_ckcv(h)
                    yield
                    for which, wsrc in (("q", wq_col), ("k", wk_col),
                                        ("v", wv_col)):
                        emit_wcol(which, wsrc, h, 0)
                        yield
                        emit_wcol(which, wsrc, h, 1)
                        yield
                        wc = wcols[(which, h)]
                        if which != "v":
                            hh = hpool.tile([128, S], bf, name="hh")
                        else:
                            hh = vpool.tile([128, S], bf, name="vv")
                        for sc in range(2):
                            pp = psA.tile([128, 512], f32, name="psp")
                            for dk in range(NDK):
                                nc.tensor.matmul(
                                    pp[:], wc[:, dk * 128:(dk + 1) * 128],
                                    xs_half(dk, sc),
                                    start=(dk == 0), stop=(dk == NDK - 1))
                                yield
                            if which == "v":
                                nc.vector.tensor_mul(
                                    hh[:, sc * 512:(sc + 1) * 512], pp[:],
                                    rsqT[:, sc * 512:(sc + 1) * 512])
                            else:
                                nc.vector.tensor_copy(
                                    hh[:, sc * 512:(sc + 1) * 512], pp[:])
                            yield
                        if which == "q":
                            emit_rope(h, "q", hh)
                            yield
                        elif which == "k":
                            emit_rope(h, "k", hh)
                            yield
                        else:
                            for tj in range(NTN):
                                ptr = psT.tile([128, 128], bf, name="ptr")
                                nc.tensor.transpose(
                                    ptr[:], hh[:, tj * 128:(tj + 1) * 128],
                                    id_t[:])
                                nc.scalar.copy(
                                    v_sb[:, h * S + tj * 128:
                                         h * S + (tj + 1) * 128], ptr[:])
                                yield

                # =================== attention head =========================
                def attn_head(h, gen, rate=5):
                    qh = qr[:, h * S:(h + 1) * S]
                    Oc = {sc: psO.tile([128, 512], f32, name="Oc")
                          for sc in range(2)}
                    acc = {}
                    cnt = {0: 0, 1: 0}
                    pend = deque()

                    def emit_epilogue(sc):
                        dv = recpool.tile([128, 512], bf, name="dv", bufs=1)
                        nc.vector.tensor_add(dv[:], acc[(sc, 0)][:],
                                             acc[(sc, 1)][:])
                        dg = recpool.tile([128, 512], bf, name="dg", bufs=1)
                        nc.vector.tensor_add(dg[:], acc[(sc, 2)][:],
                                             acc[(sc, 3)][:])
                        dc = recpool.tile([128, 512], bf, name="dc")
                        nc.vector.tensor_add(dc[:], dv[:], dg[:])
                        dps = psS.tile([128, 512], f32, name="ps")
                        nc.tensor.matmul(dps[:], ones_t[:], dc[:],
                                         start=True, stop=True)
                        rec = recpool.tile([128, 512], f32, name="rec")
                        nc.vector.reciprocal_approx_fast(rec[:], dps[:])
                        nc.vector.tensor_mul(
                            attnT[:, h * S + sc * 512:h * S + (sc + 1) * 512],
                            Oc[sc][:], rec[:])

                    def emit_pv(ti, sc, lo, e):
                        _, lv = lts_ltv(h, ti)
                        nc.tensor.matmul(Oc[sc][:, lo:512], lv, e[:, lo:512],
                                         start=(ti == pv_first[sc]),
                                         stop=(ti == pv_last[sc]))
                        if ti == pv_last[sc]:
                            emit_epilogue(sc)

                    for (ti, sc, lo, diag) in groups:
                        ls, _ = lts_ltv(h, ti)
                        ps = psS.tile([128, 512], f32, name="ps")
                        nc.tensor.matmul(
                            ps[:, lo:512], ls,
                            qh[:, sc * 512 + lo:(sc + 1) * 512],
                            start=True, stop=True)
                        e = epool.tile([128, 512], bf, name="e")
                        nc.scalar.activation(
                            e[:, lo:512], ps[:, lo:512],
                            mybir.ActivationFunctionType.Exp)
                        if diag is not None:
                            nc.vector.tensor_mul(e[:, lo:lo + 128],
                                                 e[:, lo:lo + 128], tri_t[:])
                        j = cnt[sc] % 4
                        if cnt[sc] < 4:
                            a = accpool.tile([128, 512], bf,
                                             name=f"acc_{sc}_{j}")
                            acc[(sc, j)] = a
                            nc.vector.tensor_copy(a[:], e[:])
                        else:
                            a = acc[(sc, j)]
                            nc.vector.tensor_add(a[:, lo:512], a[:, lo:512],
                                                 e[:, lo:512])
                        cnt[sc] += 1
                        if len(pend) >= 2:
                            emit_pv(*pend.popleft())
                        pend.append((ti, sc, lo, e))
                        pull(gen, rate)
                    while pend:
                        emit_pv(*pend.popleft())

                    pull(gen, 2)
                    nc.sync.dma_start(ag_in[h][:],
                                      attnT[:, h * S:(h + 1) * S])
                    ag_insts[h] = nc.gpsimd.collective_compute(
                        "AllGather", mybir.AluOpType.bypass,
                        replica_groups=[list(range(N_CORES))],
                        ins=[ag_in[h][:]], outs=[ag_out[h][:]])
                    drain(gen)

                def empty_gen():
                    return iter(())

                for h in range(3):
                    attn_head(h, qkv_units(h + 1))

            # xs/wcol/sq/hh/rope/vv pools closed: SBUF freed for gather+wo
            with (
                tc.tile_pool(name="ag", bufs=4) as agpool,
                tc.tile_pool(name="wo", bufs=4) as wopool,
            ):
                ag_sbs, wo_sbs = [], []
                for h in range(HL):
                    wo_sb = wopool.tile([128, 8 * OC], bf, name="wo_sb")
                    nc.sync.dma_start(wo_sb[:], wo_blk[h])
                    wo_sbs.append(wo_sb)
                for h in range(HL - 1):
                    ag_sb = agpool.tile([128, 8 * S], bf, name="ag_sb")
                    d = nc.sync.dma_start(
                        ag_sb[:].rearrange("p (r s) -> p r s", r=8),
                        ag_out[h][:].rearrange("(r p) s -> p r s", p=128))
                    add_dep_helper(d.ins, ag_insts[h].ins, sync=True,
                                   reason="ag_sb gather must follow AllGather")
                    ag_sbs.append(ag_sb)

                attn_head(3, empty_gen())
                ag_sb = agpool.tile([128, 8 * S], bf, name="ag_sb")
                d = nc.sync.dma_start(
                    ag_sb[:].rearrange("p (r s) -> p r s", r=8),
                    ag_out[3][:].rearrange("(r p) s -> p r s", p=128))
                add_dep_helper(d.ins, ag_insts[3].ins, sync=True,
                               reason="ag_sb gather must follow AllGather")
                ag_sbs.append(ag_sb)

                # ---- output projection y^T: h-major so the h=3 blocks
                # (gated on AllGather 3) land after 50us of h<3 matmuls;
                # per-oc8 staggered closes keep the tail to one copy+DMA ----
                psum_es.close()
                with (
                    tc.tile_pool(name="psY", bufs=4, space="PSUM") as psY,
                    tc.tile_pool(name="yout", bufs=2) as ypool,
                ):
                    ps_y = [psY.tile([128, S], f32, name="ps_y")
                            for _ in range(4)]

                    def wo_mm(oc8, sc, h, r, start, stop):
                        nc.tensor.matmul(
                            ps_y[oc8][:, sc * 512:(sc + 1) * 512],
                            wo_sbs[h][:, (r * 4 + oc8) * 128:
                                      (r * 4 + oc8 + 1) * 128],
                            ag_sbs[h][:, r * S + sc * 512:
                                      r * S + (sc + 1) * 512],
                            start=start, stop=stop)

                    for h in range(3):
                        for r in range(8):
                            for oc8 in range(4):
                                for sc in range(2):
                                    wo_mm(oc8, sc, h, r,
                                          start=(h == 0 and r == 0),
                                          stop=False)
                    for oc8 in range(4):
                        for r in range(8):
                            for sc in range(2):
                                wo_mm(oc8, sc, 3, r, start=False,
                                      stop=(r == 7))
                        ysb = ypool.tile([128, S], f32, name="ysb")
                        if oc8 % 2 == 0:
                            nc.vector.tensor_copy(ysb[:], ps_y[oc8][:])
                        else:
                            nc.scalar.copy(ysb[:], ps_y[oc8][:])
                        nc.sync.dma_start(y[oc8 * 128:(oc8 + 1) * 128, :],
                                          ysb[:])

    nc.compile()
    return nc


def _host_prep(xs, cache_k, cache_v, norm_w, wq, wk, wv, wo):
    """Build the 8 per-core input maps (all layout work done on host)."""
    xs = np.asarray(xs, F32)
    cache_k = np.asarray(cache_k, F32)
    cache_v = np.asarray(cache_v, F32)
    norm_w = np.asarray(norm_w, F32)
    wq, wk, wv, wo = (np.asarray(w, F32) for w in (wq, wk, wv, wo))

    # xs^T tiled: [128, dk*S]
    xs_t = np.ascontiguousarray(
        xs.T.reshape(NDK, 128, S).transpose(1, 0, 2).reshape(128, NDK * S)
    ).astype(BF16)

    # RoPE tables (positions C..C+S-1), transposed [freq, s]
    half = HD // 2
    inv_freq = 1.0 / (THETA ** (np.arange(0, half, dtype=np.float64) * 2.0 / HD))
    pos = np.arange(S, dtype=np.float64) + C
    ang = np.outer(pos, inv_freq)          # [S, 64]
    cos1 = np.cos(ang).T.astype(F32)       # [64, S]
    sin1 = np.sin(ang).T.astype(F32)
    cosT = np.vstack([cos1, cos1]).astype(BF16)          # [128, S]
    sinT = np.vstack([-sin1, sin1]).astype(BF16)         # rotate-half signs

    # triangle mask for diagonal blocks: tri[p, d] = 1 iff d >= p
    dd = np.arange(128)[None, :]
    pp = np.arange(128)[:, None]
    triD = (dd >= pp).astype(F32).astype(BF16)

    ones_d = np.ones((128, 128), F32).astype(BF16)
    id_d = np.eye(128, dtype=F32).astype(BF16)

    sc_q = F32(1.0) / np.sqrt(F32(HD))

    in_maps = []
    for c in range(N_CORES):
        osl = slice(OC * c, OC * (c + 1))
        hsl = slice(HL * c, HL * (c + 1))
        # fold norm_w into wq/wk/wv; fold 1/sqrt(HD) into wq
        wq_c = (wq[osl] * norm_w[None, :]) * sc_q   # [512, 4096]
        wk_c = wk[osl] * norm_w[None, :]
        wv_c = wv[osl] * norm_w[None, :]

        def col_layout(w_c):
            # [HL, 128, NDK*128]: [h, p, dk*128 + j] = w_c[h*128+j, dk*128+p]
            m = w_c.reshape(HL, 128, NDK, 128)          # [h, j, dk, p]
            return np.ascontiguousarray(
                m.transpose(0, 3, 2, 1).reshape(HL, 128, NDK * 128)).astype(BF16)

        wq_col = col_layout(wq_c)
        wk_col = col_layout(wk_c)
        wv_col = col_layout(wv_c)

        # wo block for y^T: [h, p, (r*4 + oc8)*128 + j] =
        #   wo[OC*c + oc8*128 + j, (4r + h)*128 + p]
        wo_c = wo[osl]                                  # [512, 4096]
        m = wo_c.reshape(4, 128, 8, HL, 128)            # [oc8, j, r, h, p]
        wo_blk = np.ascontiguousarray(
            m.transpose(3, 4, 2, 0, 1).reshape(HL, 128, 8 * OC)).astype(BF16)

        # cache K^T per head: [h, p(hd), t]
        ck = np.ascontiguousarray(
            cache_k[:, hsl, :].transpose(1, 2, 0)).astype(BF16)   # [HL, 128, C]
        # cache V tiles: [h, p(t%128), ti*128 + hd]
        cv = np.ascontiguousarray(
            cache_v[:, hsl, :].reshape(NTC, 128, HL, HD)
            .transpose(2, 1, 0, 3).reshape(HL, 128, C)).astype(BF16)

        in_maps.append({
            "xs_t": xs_t, "wq_col": wq_col, "wk_col": wk_col, "wv_col": wv_col,
            "wo_blk": wo_blk, "ckt": ck, "cvr": cv,
            "cosT": cosT, "sinT": sinT, "triD": triD,
            "ones_d": ones_d, "id_d": id_d,
        })
    return in_maps


_NC_CACHE = {}


def kernel(xs, cache_k, cache_v, norm_w, wq, wk, wv, wo, _trace=False):
    if "nc" not in _NC_CACHE:
        _NC_CACHE["nc"] = _build_nc()
    nc = _NC_CACHE["nc"]
    in_maps = _host_prep(xs, cache_k, cache_v, norm_w, wq, wk, wv, wo)
    res = bass_utils.run_bass_kernel_spmd(
        nc, in_maps, core_ids=list(range(N_CORES)), trace=_trace)
    out = np.concatenate(
        [res.results[c]["y"].T for c in range(N_CORES)], axis=1)
    out = np.ascontiguousarray(out)
    if _trace:
        kernel.last_exec_time_ns = res.exec_time_ns
        kernel.last_results = res
    return out


# revision 20
# speedup vs baseline: 1.0040x; 1.0040x over previous
"""Trainium2 Bass kernel for a single-layer attention module (RMSNorm + QKV +
RoPE + causal attention over a KV cache + output projection), tensor-parallel
over 8 NeuronCores (4 heads each), per-head AllGather of attention outputs,
and per-core output-column blocks of the final projection.

Head-interleaved schedule: the QKV projection matmuls of head h+1 are emitted
interleaved into the attention groups of head h so the PE never waits on the
scalar engine's exp. The softmax denominator is accumulated on the vector
engine (4 bf16 sub-accumulators per (head, sc)) and collapsed with a single
ones-matmul per (head, sc) instead of one denominator matmul per tile.
Causal work on the new-key tiles is restricted to the unmasked column range,
with a single constant 128x128 triangle mask for the diagonal blocks.

Self-contained: takes FULL inputs, returns the FULL [1024, 4096] f32 output.
"""

import sys
from collections import deque
from contextlib import ExitStack

sys.path.insert(0, "/opt/trn_rl_repo")

import numpy as np
import ml_dtypes

import concourse.bass as bass  # noqa: F401
import concourse.bacc as bacc
import concourse.tile as tile
from concourse.tile_rust import add_dep_helper
from concourse import mybir
from concourse import bass_utils

BF16 = ml_dtypes.bfloat16
F32 = np.float32

N_CORES = 8
D, H, HD, S, C = 4096, 32, 128, 1024, 2048
T = C + S          # 3072 total keys
HL = H // N_CORES  # 4 heads per core
OC = HL * HD       # 512 local attention features per core
NDK = D // 128     # 32 contraction tiles over D
NTC = C // 128     # 16 cache t-tiles
NTN = S // 128     # 8 new-key t-tiles
NT = NTC + NTN     # 24 t-tiles
EPS = 1e-6
THETA = 10000.0

bf = mybir.dt.bfloat16
f32 = mybir.dt.float32


def _attn_groups():
    """Group list per head: (ti, sc, lo, diag).

    lo = first valid column within the sc-half (columns < lo fully masked).
    diag = column (== lo) where a 128-wide triangle mask applies, or None.
    """
    out = []
    for ti in range(NT):
        for sc in range(2):
            if ti < NTC:
                out.append((ti, sc, 0, None))
                continue
            tn = ti - NTC
            dsc = tn // 4                      # half containing the diagonal
            if sc < dsc:
                continue                       # fully masked
            if sc == dsc:
                lo = 128 * (tn % 4)
                out.append((ti, sc, lo, lo))
            else:
                out.append((ti, sc, 0, None))
    return out


def _build_nc():
    nc = bacc.Bacc("TRN2", target_bir_lowering=False, debug=False,
                   num_devices=N_CORES)

    # ---- DRAM I/O ----
    xs_t = nc.dram_tensor("xs_t", [128, NDK * S], bf, kind="ExternalInput")
    wq_col = nc.dram_tensor("wq_col", [HL, 128, NDK * 128], bf, kind="ExternalInput")
    wk_col = nc.dram_tensor("wk_col", [HL, 128, NDK * 128], bf, kind="ExternalInput")
    wv_col = nc.dram_tensor("wv_col", [HL, 128, NDK * 128], bf, kind="ExternalInput")
    wo_blk = nc.dram_tensor("wo_blk", [HL, 128, 8 * OC], bf, kind="ExternalInput")
    ckt = nc.dram_tensor("ckt", [HL, 128, C], bf, kind="ExternalInput")
    cvr = nc.dram_tensor("cvr", [HL, 128, C], bf, kind="ExternalInput")
    cosT = nc.dram_tensor("cosT", [128, S], bf, kind="ExternalInput")
    sinT = nc.dram_tensor("sinT", [128, S], bf, kind="ExternalInput")
    triD = nc.dram_tensor("triD", [128, 128], bf, kind="ExternalInput")
    ones_d = nc.dram_tensor("ones_d", [128, 128], bf, kind="ExternalInput")
    id_d = nc.dram_tensor("id_d", [128, 128], bf, kind="ExternalInput")
    # y stored transposed ([outcol, s]); host transposes back
    y = nc.dram_tensor("y", [OC, S], f32, kind="ExternalOutput")
    DEBUG = False
    if DEBUG:
        dbg_qr = nc.dram_tensor("dbg_qr", [128, HL * S], bf, kind="ExternalOutput")
        dbg_kr = nc.dram_tensor("dbg_kr", [128, HL * S], bf, kind="ExternalOutput")
        dbg_v = nc.dram_tensor("dbg_v", [128, HL * S], bf, kind="ExternalOutput")
        dbg_at = nc.dram_tensor("dbg_at", [128, HL * S], bf, kind="ExternalOutput")
        dbg_rsq = nc.dram_tensor("dbg_rsq", [128, S], bf, kind="ExternalOutput")
        dbg_ag = [nc.dram_tensor(f"dbg_ag{h}", [128, 8 * S], bf,
                  kind="ExternalOutput") for h in range(HL)]

    groups = _attn_groups()
    # first/last participating group per sc (for PV start/stop flags)
    part = {sc: [g for g in groups if g[1] == sc] for sc in range(2)}
    pv_first = {sc: part[sc][0][0] for sc in range(2)}
    pv_last = {sc: part[sc][-1][0] for sc in range(2)}

    with tile.TileContext(nc) as tc:
        with (
            tc.tile_pool(name="const", bufs=1) as cpool,
            tc.tile_pool(name="qk", bufs=1) as qkpool,
            tc.tile_pool(name="kv", bufs=4) as kvpool,
            tc.tile_pool(name="exp", bufs=6) as epool,
            tc.tile_pool(name="acc", bufs=1) as accpool,
            tc.tile_pool(name="rec", bufs=2) as recpool,
            tc.tile_pool(name="dram", bufs=1, space="DRAM") as dpool,
        ):
            # persistent per-head results
            qr = qkpool.tile([128, HL * S], bf, name="qr")
            kr = qkpool.tile([128, HL * S], bf, name="kr")
            v_sb = qkpool.tile([128, HL * S], bf, name="v_sb")
            attnT = qkpool.tile([128, HL * S], bf, name="attnT")
            rsqT = qkpool.tile([128, S], bf, name="rsqT")
            cosp = qkpool.tile([128, S], bf, name="cosp")
            sinp = qkpool.tile([128, S], bf, name="sinp")
            ag_in = [dpool.tile([128, S], bf, name=f"ag_in{h}") for h in range(HL)]
            ag_out = [dpool.tile([N_CORES * 128, S], bf, name=f"ag_out{h}",
                                 addr_space="Shared") for h in range(HL)]

            psum_es = ExitStack()
            psA = psum_es.enter_context(
                tc.tile_pool(name="psA", bufs=2, space="PSUM"))
            psS = psum_es.enter_context(
                tc.tile_pool(name="psS", bufs=2, space="PSUM"))
            psO = psum_es.enter_context(
                tc.tile_pool(name="psO", bufs=2, space="PSUM"))
            psT = psum_es.enter_context(
                tc.tile_pool(name="psT", bufs=2, space="PSUM"))

            head_kv = {}   # h -> (ck_sb, cv_sb)
            ag_insts = {}  # h -> collective BassInstruction

            def lts_ltv(h, ti):
                ck_sb, cv_sb = head_kv[h]
                if ti < NTC:
                    return (ck_sb[:, ti * 128:(ti + 1) * 128],
                            cv_sb[:, ti * 128:(ti + 1) * 128])
                tn = ti - NTC
                return (kr[:, h * S + tn * 128: h * S + (tn + 1) * 128],
                        v_sb[:, h * S + tn * 128: h * S + (tn + 1) * 128])

            INTERLEAVE = True

            def pull(gen, n):
                if not INTERLEAVE:
                    return False
                for _ in range(n):
                    if next(gen, None) is None:
                        return False
                return True

            def drain(gen):
                for _ in gen:
                    pass

            with (
                tc.tile_pool(name="xs", bufs=32) as xpool,
                tc.tile_pool(name="sq", bufs=2) as sqpool,
                tc.tile_pool(name="wcol", bufs=2) as wpool,
                tc.tile_pool(name="hh", bufs=2) as hpool,
                tc.tile_pool(name="rope", bufs=2) as rpool,
                tc.tile_pool(name="vv", bufs=2) as vpool,
            ):
                # ---- tiny constants + xs, ordered so PE starts ~2us in ----
                ones_t = cpool.tile([128, 128], bf, name="ones_t")
                nc.sync.dma_start(ones_t[:], ones_d[:])
                xs_ch = []
                xc = xpool.tile([128, S], bf, name="xs_ch")
                nc.sync.dma_start(xc[:], xs_t[:, 0:S])
                xs_ch.append(xc)

                wcols = {}

                def emit_wcol(which, wsrc, h, half):
                    key = (which, h)
                    if half == 0:
                        wcols[key] = wpool.tile([128, NDK * 128], bf, name="wcol")
                    halfsz = NDK * 64
                    nc.sync.dma_start(
                        wcols[key][:, half * halfsz:(half + 1) * halfsz],
                        wsrc[h][:, half * halfsz:(half + 1) * halfsz])

                emit_wcol("q", wq_col, 0, 0)
                xc = xpool.tile([128, S], bf, name="xs_ch")
                nc.sync.dma_start(xc[:], xs_t[:, S:2 * S])
                xs_ch.append(xc)
                emit_wcol("q", wq_col, 0, 1)
                emit_wcol("k", wk_col, 0, 0)
                emit_wcol("k", wk_col, 0, 1)
                for dk in range(2, 6):
                    xc = xpool.tile([128, S], bf, name="xs_ch")
                    nc.sync.dma_start(xc[:], xs_t[:, dk * S:(dk + 1) * S])
                    xs_ch.append(xc)
                cos_t = cpool.tile([128, S], bf, name="cos_t")
                nc.sync.dma_start(cos_t[:], cosT[:])
                sin_t = cpool.tile([128, S], bf, name="sin_t")
                nc.sync.dma_start(sin_t[:], sinT[:])
                id_t = cpool.tile([128, 128], bf, name="id_t")
                nc.sync.dma_start(id_t[:], id_d[:])
                tri_t = cpool.tile([128, 128], bf, name="tri_t")
                nc.sync.dma_start(tri_t[:], triD[:])
                for dk in range(6, NDK):
                    xc = xpool.tile([128, S], bf, name="xs_ch")
                    nc.sync.dma_start(xc[:], xs_t[:, dk * S:(dk + 1) * S])
                    xs_ch.append(xc)

                def xs_half(dk, sc):
                    return xs_ch[dk][:, sc * 512:(sc + 1) * 512]

                def emit_ckcv(h):
                    ck_sb = kvpool.tile([128, C], bf, name="ck_sb")
                    nc.sync.dma_start(ck_sb[:], ckt[h])
                    cv_sb = kvpool.tile([128, C], bf, name="cv_sb")
                    nc.sync.dma_start(cv_sb[:], cvr[h])
                    head_kv[h] = (ck_sb, cv_sb)

                def emit_rope(h, which, hh):
                    # dst = hh*cos' + rot(hh)*sin' where cos'/sin' fold rsq
                    rot = rpool.tile([128, S], bf, name="rot")
                    nc.sync.dma_start(rot[0:64, :], hh[64:128, :])
                    nc.sync.dma_start(rot[64:128, :], hh[0:64, :])
                    ta = rpool.tile([128, S], bf, name="ta", bufs=1)
                    nc.vector.tensor_mul(ta[:], hh[:], cosp[:])
                    tb = rpool.tile([128, S], bf, name="tb", bufs=1)
                    nc.vector.tensor_mul(tb[:], rot[:], sinp[:])
                    dst = qr if which == "q" else kr
                    nc.vector.tensor_add(dst[:, h * S:(h + 1) * S], ta[:], tb[:])

                # ================= head 0 QKV (inline, with RMSNorm) ========
                hh_q = hpool.tile([128, S], bf, name="hh")
                stats = {sc: psS.tile([128, 512], f32, name="ps")
                         for sc in range(2)}
                qps = {sc: psA.tile([128, 512], f32, name="psp")
                       for sc in range(2)}
                kps = {sc: psO.tile([128, 512], f32, name="Oc")
                       for sc in range(2)}
                wc = wcols[("q", 0)]
                wck = wcols[("k", 0)]
                nacc = None
                for dk in range(NDK):
                    # squares accumulated 4 dk-tiles at a time on vector,
                    # then one ones-matmul per (group-of-4, sc) on the PE
                    if dk % 4 == 0:
                        nacc = sqpool.tile([128, S], bf, name="nacc", bufs=2)
                        nc.vector.tensor_mul(nacc[:], xs_ch[dk][:],
                                             xs_ch[dk][:])
                    else:
                        sqt = sqpool.tile([128, S], bf, name="sqt", bufs=2)
                        nc.vector.tensor_mul(sqt[:], xs_ch[dk][:],
                                             xs_ch[dk][:])
                        nc.vector.tensor_add(nacc[:], nacc[:], sqt[:])
                    for sc in range(2):
                        if dk % 4 == 3:
                            nc.tensor.matmul(stats[sc][:], ones_t[:],
                                             nacc[:, sc * 512:(sc + 1) * 512],
                                             start=(dk == 3),
                                             stop=(dk == NDK - 1))
                        nc.tensor.matmul(qps[sc][:],
                                         wc[:, dk * 128:(dk + 1) * 128],
                                         xs_half(dk, sc),
                                         start=(dk == 0), stop=(dk == NDK - 1))
                        nc.tensor.matmul(kps[sc][:],
                                         wck[:, dk * 128:(dk + 1) * 128],
                                         xs_half(dk, sc),
                                         start=(dk == 0), stop=(dk == NDK - 1))
                for sc in range(2):
                    # rsq half + folded rope tables
                    ssum = recpool.tile([128, 512], f32, name="ssum", bufs=1)
                    nc.scalar.activation(ssum[:], stats[sc][:],
                                         mybir.ActivationFunctionType.Copy,
                                         bias=EPS, scale=1.0 / D)
                    rcp = recpool.tile([128, 512], f32, name="rcp", bufs=1)
                    nc.vector.reciprocal_approx_fast(rcp[:], ssum[:])
                    nc.scalar.sqrt(rsqT[:, sc * 512:(sc + 1) * 512], rcp[:])
                    nc.vector.tensor_mul(cosp[:, sc * 512:(sc + 1) * 512],
                                         cos_t[:, sc * 512:(sc + 1) * 512],
                                         rsqT[:, sc * 512:(sc + 1) * 512])
                    nc.vector.tensor_mul(sinp[:, sc * 512:(sc + 1) * 512],
                                         sin_t[:, sc * 512:(sc + 1) * 512],
                                         rsqT[:, sc * 512:(sc + 1) * 512])
                    nc.vector.tensor_copy(hh_q[:, sc * 512:(sc + 1) * 512],
                                          qps[sc][:])

                emit_ckcv(0)
                hh_k = hpool.tile([128, S], bf, name="hh")
                for sc in range(2):
                    nc.vector.tensor_copy(hh_k[:, sc * 512:(sc + 1) * 512],
                                          kps[sc][:])
                emit_rope(0, "q", hh_q)
                emit_rope(0, "k", hh_k)

                emit_wcol("v", wv_col, 0, 0)
                emit_wcol("v", wv_col, 0, 1)
                vvt = vpool.tile([128, S], bf, name="vv")
                for sc in range(2):
                    vp = psA.tile([128, 512], f32, name="psp")
                    wc = wcols[("v", 0)]
                    for dk in range(NDK):
                        nc.tensor.matmul(vp[:], wc[:, dk * 128:(dk + 1) * 128],
                                         xs_half(dk, sc),
                                         start=(dk == 0), stop=(dk == NDK - 1))
                    nc.vector.tensor_mul(vvt[:, sc * 512:(sc + 1) * 512], vp[:],
                                         rsqT[:, sc * 512:(sc + 1) * 512])
                for tj in range(NTN):
                    ptr = psT.tile([128, 128], bf, name="ptr")
                    nc.tensor.transpose(ptr[:], vvt[:, tj * 128:(tj + 1) * 128],
                                        id_t[:])
                    nc.scalar.copy(v_sb[:, tj * 128:(tj + 1) * 128], ptr[:])

                # ============ QKV emission units for heads 1..3 =============
                def qkv_units(h):
                    emit_ckcv(h)
                    yield
                    for which, wsrc in (("q", wq_col), ("k", wk_col),
                                        ("v", wv_col)):
                        emit_wcol(which, wsrc, h, 0)
                        yield
                        emit_wcol(which, wsrc, h, 1)
                        yield
                        wc = wcols[(which, h)]
                        if which != "v":
                            hh = hpool.tile([128, S], bf, name="hh")
                        else:
                            hh = vpool.tile([128, S], bf, name="vv")
                        for sc in range(2):
                            pp = psA.tile([128, 512], f32, name="psp")
                            for dk in range(NDK):
                                nc.tensor.matmul(
                                    pp[:], wc[:, dk * 128:(dk + 1) * 128],
                                    xs_half(dk, sc),
                                    start=(dk == 0), stop=(dk == NDK - 1))
                                yield
                            if which == "v":
                                nc.vector.tensor_mul(
                                    hh[:, sc * 512:(sc + 1) * 512], pp[:],
                                    rsqT[:, sc * 512:(sc + 1) * 512])
                            else:
                                nc.vector.tensor_copy(
                                    hh[:, sc * 512:(sc + 1) * 512], pp[:])
                            yield
                        if which == "q":
                            emit_rope(h, "q", hh)
                            yield
                        elif which == "k":
                            emit_rope(h, "k", hh)
                            yield
                        else:
                            for tj in range(NTN):
                                ptr = psT.tile([128, 128], bf, name="ptr")
                                nc.tensor.transpose(
                                    ptr[:], hh[:, tj * 128:(tj + 1) * 128],
                                    id_t[:])
                                nc.scalar.copy(
                                    v_sb[:, h * S + tj * 128:
                                         h * S + (tj + 1) * 128], ptr[:])
                                yield

                # =================== attention head =========================
                def attn_head(h, gen, rate=5):
                    qh = qr[:, h * S:(h + 1) * S]
                    Oc = {sc: psO.tile([128, 512], f32, name="Oc")
                          for sc in range(2)}
                    acc = {}
                    cnt = {0: 0, 1: 0}
                    pend = deque()

                    def emit_epilogue(sc):
                        dv = recpool.tile([128, 512], bf, name="dv", bufs=1)
                        nc.vector.tensor_add(dv[:], acc[(sc, 0)][:],
                                             acc[(sc, 1)][:])
                        dg = recpool.tile([128, 512], bf, name="dg", bufs=1)
                        nc.vector.tensor_add(dg[:], acc[(sc, 2)][:],
                                             acc[(sc, 3)][:])
                        dc = recpool.tile([128, 512], bf, name="dc")
                        nc.vector.tensor_add(dc[:], dv[:], dg[:])
                        dps = psS.tile([128, 512], f32, name="ps")
                        nc.tensor.matmul(dps[:], ones_t[:], dc[:],
                                         start=True, stop=True)
                        rec = recpool.tile([128, 512], f32, name="rec")
                        nc.vector.reciprocal_approx_fast(rec[:], dps[:])
                        nc.vector.tensor_mul(
                            attnT[:, h * S + sc * 512:h * S + (sc + 1) * 512],
                            Oc[sc][:], rec[:])

                    def emit_pv(ti, sc, lo, e):
                        _, lv = lts_ltv(h, ti)
                        nc.tensor.matmul(Oc[sc][:, lo:512], lv, e[:, lo:512],
                                         start=(ti == pv_first[sc]),
                                         stop=(ti == pv_last[sc]))
                        if ti == pv_last[sc]:
                            emit_epilogue(sc)

                    for (ti, sc, lo, diag) in groups:
                        ls, _ = lts_ltv(h, ti)
                        ps = psS.tile([128, 512], f32, name="ps")
                        nc.tensor.matmul(
                            ps[:, lo:512], ls,
                            qh[:, sc * 512 + lo:(sc + 1) * 512],
                            start=True, stop=True)
                        e = epool.tile([128, 512], bf, name="e")
                        nc.scalar.activation(
                            e[:, lo:512], ps[:, lo:512],
                            mybir.ActivationFunctionType.Exp)
                        if diag is not None:
                            nc.vector.tensor_mul(e[:, lo:lo + 128],
                                                 e[:, lo:lo + 128], tri_t[:])
                        j = cnt[sc] % 4
                        if cnt[sc] < 4:
                            a = accpool.tile([128, 512], bf,
                                             name=f"acc_{sc}_{j}")
                            acc[(sc, j)] = a
                            nc.vector.tensor_copy(a[:], e[:])
                        else:
                            a = acc[(sc, j)]
                            nc.vector.tensor_add(a[:, lo:512], a[:, lo:512],
                                                 e[:, lo:512])
                        cnt[sc] += 1
                        if len(pend) >= 2:
                            emit_pv(*pend.popleft())
                        pend.append((ti, sc, lo, e))
                        pull(gen, rate)
                    while pend:
                        emit_pv(*pend.popleft())

                    pull(gen, 2)
                    nc.sync.dma_start(ag_in[h][:],
                                      attnT[:, h * S:(h + 1) * S])
                    ag_insts[h] = nc.gpsimd.collective_compute(
                        "AllGather", mybir.AluOpType.bypass,
                        replica_groups=[list(range(N_CORES))],
                        ins=[ag_in[h][:]], outs=[ag_out[h][:]])
                    drain(gen)

                def empty_gen():
                    return iter(())

                for h in range(3):
                    attn_head(h, qkv_units(h + 1))

            # xs/wcol/sq/hh/rope/vv pools closed: SBUF freed for gather+wo
            with (
                tc.tile_pool(name="ag", bufs=4) as agpool,
                tc.tile_pool(name="wo", bufs=4) as wopool,
            ):
                ag_sbs, wo_sbs = [], []
                for h in range(HL):
                    wo_sb = wopool.tile([128, 8 * OC], bf, name="wo_sb")
                    nc.sync.dma_start(wo_sb[:], wo_blk[h])
                    wo_sbs.append(wo_sb)
                for h in range(HL - 1):
                    ag_sb = agpool.tile([128, 8 * S], bf, name="ag_sb")
                    d = nc.sync.dma_start(
                        ag_sb[:].rearrange("p (r s) -> p r s", r=8),
                        ag_out[h][:].rearrange("(r p) s -> p r s", p=128))
                    add_dep_helper(d.ins, ag_insts[h].ins, sync=True,
                                   reason="ag_sb gather must follow AllGather")
                    ag_sbs.append(ag_sb)

                attn_head(3, empty_gen())
                ag_sb = agpool.tile([128, 8 * S], bf, name="ag_sb")
                d = nc.sync.dma_start(
                    ag_sb[:].rearrange("p (r s) -> p r s", r=8),
                    ag_out[3][:].rearrange("(r p) s -> p r s", p=128))
                add_dep_helper(d.ins, ag_insts[3].ins, sync=True,
                               reason="ag_sb gather must follow AllGather")
                ag_sbs.append(ag_sb)

                # ---- output projection y^T: h-major so the h=3 blocks
                # (gated on AllGather 3) land after 50us of h<3 matmuls;
                # per-oc8 staggered closes keep the tail to one copy+DMA ----
                psum_es.close()
                with (
                    tc.tile_pool(name="psY", bufs=4, space="PSUM") as psY,
                    tc.tile_pool(name="yout", bufs=2) as ypool,
                ):
                    ps_y = [psY.tile([128, S], f32, name="ps_y")
                            for _ in range(4)]

                    def wo_mm(oc8, sc, h, r, start, stop):
                        nc.tensor.matmul(
                            ps_y[oc8][:, sc * 512:(sc + 1) * 512],
                            wo_sbs[h][:, (r * 4 + oc8) * 128:
                                      (r * 4 + oc8 + 1) * 128],
                            ag_sbs[h][:, r * S + sc * 512:
                                      r * S + (sc + 1) * 512],
                            start=start, stop=stop)

                    for h in range(3):
                        for r in range(8):
                            for oc8 in range(4):
                                for sc in range(2):
                                    wo_mm(oc8, sc, h, r,
                                          start=(h == 0 and r == 0),
                                          stop=False)
                    for oc8 in range(4):
                        for r in range(8):
                            for sc in range(2):
                                wo_mm(oc8, sc, 3, r, start=False,
                                      stop=(r == 7))
                        ysb = ypool.tile([128, S], f32, name="ysb")
                        if oc8 % 2 == 0:
                            nc.vector.tensor_copy(ysb[:], ps_y[oc8][:])
                        else:
                            nc.scalar.copy(ysb[:], ps_y[oc8][:])
                        nc.sync.dma_start(y[oc8 * 128:(oc8 + 1) * 128, :],
                                          ysb[:])

    nc.compile()
    return nc


def _host_prep(xs, cache_k, cache_v, norm_w, wq, wk, wv, wo):
    """Build the 8 per-core input maps (all layout work done on host)."""
    xs = np.asarray(xs, F32)
    cache_k = np.asarray(cache_k, F32)
    cache_v = np.asarray(cache_v, F32)
    norm_w = np.asarray(norm_w, F32)
    wq, wk, wv, wo = (np.asarray(w, F32) for w in (wq, wk, wv, wo))

    # xs^T tiled: [128, dk*S]
    xs_t = np.ascontiguousarray(
        xs.T.reshape(NDK, 128, S).transpose(1, 0, 2).reshape(128, NDK * S)
    ).astype(BF16)

    # RoPE tables (positions C..C+S-1), transposed [freq, s]
    half = HD // 2
    inv_freq = 1.0 / (THETA ** (np.arange(0, half, dtype=np.float64) * 2.0 / HD))
    pos = np.arange(S, dtype=np.float64) + C
    ang = np.outer(pos, inv_freq)          # [S, 64]
    cos1 = np.cos(ang).T.astype(F32)       # [64, S]
    sin1 = np.sin(ang).T.astype(F32)
    cosT = np.vstack([cos1, cos1]).astype(BF16)          # [128, S]
    sinT = np.vstack([-sin1, sin1]).astype(BF16)         # rotate-half signs

    # triangle mask for diagonal blocks: tri[p, d] = 1 iff d >= p
    dd = np.arange(128)[None, :]
    pp = np.arange(128)[:, None]
    triD = (dd >= pp).astype(F32).astype(BF16)

    ones_d = np.ones((128, 128), F32).astype(BF16)
    id_d = np.eye(128, dtype=F32).astype(BF16)

    sc_q = F32(1.0) / np.sqrt(F32(HD))

    in_maps = []
    for c in range(N_CORES):
        osl = slice(OC * c, OC * (c + 1))
        hsl = slice(HL * c, HL * (c + 1))
        # fold norm_w into wq/wk/wv; fold 1/sqrt(HD) into wq
        wq_c = (wq[osl] * norm_w[None, :]) * sc_q   # [512, 4096]
        wk_c = wk[osl] * norm_w[None, :]
        wv_c = wv[osl] * norm_w[None, :]

        def col_layout(w_c):
            # [HL, 128, NDK*128]: [h, p, dk*128 + j] = w_c[h*128+j, dk*128+p]
            m = w_c.reshape(HL, 128, NDK, 128)          # [h, j, dk, p]
            return np.ascontiguousarray(
                m.transpose(0, 3, 2, 1).reshape(HL, 128, NDK * 128)).astype(BF16)

        wq_col = col_layout(wq_c)
        wk_col = col_layout(wk_c)
        wv_col = col_layout(wv_c)

        # wo block for y^T: [h, p, (r*4 + oc8)*128 + j] =
        #   wo[OC*c + oc8*128 + j, (4r + h)*128 + p]
        wo_c = wo[osl]                                  # [512, 4096]
        m = wo_c.reshape(4, 128, 8, HL, 128)            # [oc8, j, r, h, p]
        wo_blk = np.ascontiguousarray(
            m.transpose(3, 4, 2, 0, 1).reshape(HL, 128, 8 * OC)).astype(BF16)

        # cache K^T per head: [h, p(hd), t]
        ck = np.ascontiguousarray(
            cache_k[:, hsl, :].transpose(1, 2, 0)).astype(BF16)   # [HL, 128, C]
        # cache V tiles: [h, p(t%128), ti*128 + hd]
        cv = np.ascontiguousarray(
            cache_v[:, hsl, :].reshape(NTC, 128, HL, HD)
            .transpose(2, 1, 0, 3).reshape(HL, 128, C)).astype(BF16)

        in_maps.append({
            "xs_t": xs_t, "wq_col": wq_col, "wk_col": wk_col, "wv_col": wv_col,
            "wo_blk": wo_blk, "ckt": ck, "cvr": cv,
            "cosT": cosT, "sinT": sinT, "triD": triD,
            "ones_d": ones_d, "id_d": id_d,
        })
    return in_maps


_NC_CACHE = {}


def kernel(xs, cache_k, cache_v, norm_w, wq, wk, wv, wo, _trace=False):
    if "nc" not in _NC_CACHE:
        _NC_CACHE["nc"] = _build_nc()
    nc = _NC_CACHE["nc"]
    in_maps = _host_prep(xs, cache_k, cache_v, norm_w, wq, wk, wv, wo)
    res = bass_utils.run_bass_kernel_spmd(
        nc, in_maps, core_ids=list(range(N_CORES)), trace=_trace)
    out = np.concatenate(
        [res.results[c]["y"].T for c in range(N_CORES)], axis=1)
    out = np.ascontiguousarray(out)
    if _trace:
        kernel.last_exec_time_ns = res.exec_time_ns
        kernel.last_results = res
    return out
